# revision 1
# baseline (speedup 1.0000x reference)
"""Grouped-correlation cost volume (CostVolume) Bass kernel for Trainium2.

Problem: x, y: (4, 512, 128, 256) f32; GROUP=4, MAXDISP=48, D=49.
out[b, g, k, h, w] = sum_cg x[b, 128g+cg, h, w] * y[b, 128g+cg, h, w-k]
(zero where w < k), out shape (4, 4, 49, 128, 256).

Strategy: shard the 16 (b, g) units over 8 cores (2 each; the channel sum is
within-group, so no cross-core reduce). Per (unit, h) row the correlation is
a banded Gram matrix between x columns and y columns with contraction over
cg = 128 = the TensorE partition dim. To keep the stored band nearly
rectangular, each 128-wide w-block is split into four M=32 column groups
(tile_position col-tiling) whose y-windows are shifted by the group base:

  P[32m+i', 80t+j'] = sum_cg x[cg, 128t+32m+i'] * ypad[cg, 128t+32m-48+j']

so the useful entries are j' = i' + 48 - k with i' in [0,32), j' in [0,80) —
a 32x80 parallelogram per group (1.63x amplification instead of 3.6x for
M=128). The per-(unit,h) (128, 160) PSUM tile is copied to SBUF and stored
to DRAM as-is; the band extraction (a pure gather) happens on the host
during the unshard step.

The module is built through bacc (not raw bass) so excess semaphore waits
get split onto EventSemaphore instructions — TRN2 allows at most one sync
wait per regular instruction.
"""

import os

import numpy as np

import concourse.bass as bass
import concourse.mybir as mybir
import concourse.tile as tile
from concourse import bacc

MAXDISP = 48
D = MAXDISP + 1          # 49 disparities
CG = 128                 # channels per group = contraction dim
GROUP = 4
B = 4
H = 128
W = 256
NB = W // 128            # 2 w-blocks of 128
NM = 4                   # M=32 col groups per w-block
MW = 32                  # group width
NWIN = MAXDISP + MW      # 80: y window per group
N_CORES = 8
N_UNITS = 2              # (b,g) units per core
ROW = 384                # padded y row: 48 left pad + 256 data + 80 slack
NY_BUFS = 3

_last_results = None     # BassKernelResults of the most recent run (for test.py)


def build_nc(n_units=N_UNITS, n_h=H, h_chunk=16):
    """Build the per-core Bass module (fp32)."""
    assert n_h % h_chunk == 0
    n_chunks = n_h // h_chunk
    f32 = mybir.dt.float32
    rect_w = NB * NWIN   # 160 columns per (unit, h)

    nc = bacc.Bacc()
    x = nc.dram_tensor("x", [n_units, CG, n_h, W], f32, kind="ExternalInput")
    y = nc.dram_tensor("y", [n_units, CG, n_h, W], f32, kind="ExternalInput")
    out = nc.dram_tensor(
        "out", [n_units, n_chunks, 128, h_chunk, rect_w], f32, kind="ExternalOutput"
    )

    y_flat_len = ROW * h_chunk + 48  # slack so the pad memset can cover tails

    with tile.TileContext(nc) as tc:
        with (
            tc.tile_pool(name="io", bufs=2) as io_pool,
            tc.tile_pool(name="ybufs", bufs=1) as y_pool,
            tc.tile_pool(name="work", bufs=2) as work_pool,
            tc.tile_pool(name="psum_mm", bufs=4, space="PSUM") as psum_mm,
        ):
            # persistent y buffers with pads zeroed once (DVE so the pad
            # dependency rides the DVE semaphore, not Pool)
            y_bufs = []
            for i in range(NY_BUFS):
                yb = y_pool.tile([128, y_flat_len], f32, name=f"y_buf{i}")
                nc.vector.memset(yb[:, 0:48], 0.0)
                pad2 = bass.AP(
                    tensor=yb.tensor,
                    offset=yb.offset + 304,
                    ap=[[y_flat_len, 128], [ROW, h_chunk], [1, 128]],
                )
                nc.vector.memset(pad2, 0.0)
                y_bufs.append(yb)

            for u in range(n_units):
                for hc in range(n_chunks):
                    h0 = hc * h_chunk
                    x_tile = io_pool.tile([128, h_chunk, W], f32, name="x_tile", tag="x")
                    nc.sync.dma_start(out=x_tile, in_=x[u, :, h0 : h0 + h_chunk, :])

                    y_tile = y_bufs[(u * n_chunks + hc) % NY_BUFS]
                    # y rows land at [h*ROW + 48, h*ROW + 304)
                    y_dst = bass.AP(
                        tensor=y_tile.tensor,
                        offset=y_tile.offset + 48,
                        ap=[[y_flat_len, 128], [ROW, h_chunk], [1, W]],
                    )
                    nc.scalar.dma_start(out=y_dst, in_=y[u, :, h0 : h0 + h_chunk, :])

                    # per-chunk staging tile so the store is one big DMA
                    s_big = work_pool.tile(
                        [128, h_chunk, rect_w], f32, name="s_big", tag="S"
                    )
                    for h in range(h_chunk):
                        p_mm = psum_mm.tile([128, rect_w], f32, name="p_mm", tag="P")
                        for t in range(NB):
                            for m in range(NM):
                                base = 128 * t + MW * m
                                lhsT = x_tile[:, h, base : base + MW]
                                # tile coords: ypad[w2] at 48 + w2
                                rhs = y_tile[:, h * ROW + base : h * ROW + base + NWIN]
                                nc.tensor.matmul(
                                    p_mm[MW * m : MW * (m + 1),
                                         NWIN * t : NWIN * (t + 1)],
                                    lhsT,
                                    rhs,
                                    start=True,
                                    stop=True,
                                    tile_position=(0, MW * m),
                                )
                        nc.vector.tensor_copy(s_big[:, h, :], p_mm)
                    # chunk-major out layout: 10 KB contiguous per partition
                    st_eng = nc.sync if (hc % 2 == 0) else nc.scalar
                    st_eng.dma_start(out=out[u, hc], in_=s_big)

    nc.finalize()
    return nc


def _shard_inputs(x, y):
    """x, y: (4, 512, 128, 256) -> per-core dicts of (2, 128, 128, 256)."""
    xu = x.reshape(B * GROUP, CG, H, W)
    yu = y.reshape(B * GROUP, CG, H, W)
    in_maps = []
    for c in range(N_CORES):
        in_maps.append(
            {
                "x": np.ascontiguousarray(xu[2 * c : 2 * c + 2]),
                "y": np.ascontiguousarray(yu[2 * c : 2 * c + 2]),
            }
        )
    return in_maps


def _extract_band(rect, n_h=H):
    """rect: (n, n_chunks, 128, h_chunk, 160) rects -> (n, D, n_h, W) volume.

    rect[n, hc, 32m+i, h', 80t+j] = out[n, 48-(j-i), hc*h_chunk+h', 128t+32m+i]
    for j-i in [0, 48].
    """
    n, n_chunks, _, h_chunk, _ = rect.shape
    r = rect.reshape(n, n_chunks, NM, MW, h_chunk, NB, NWIN)  # [n,hc,m,i,h',t,j]
    idx = np.arange(MW)[:, None] + np.arange(D)[None, :]      # j = i + c
    g = np.take_along_axis(
        r, idx[None, None, None, :, None, None, :], axis=-1
    )  # [n, hc, m, i, h', t, c]
    g = g.transpose(0, 6, 1, 4, 5, 2, 3)                      # [n,c,hc,h',t,m,i]
    g = g.reshape(n, D, n_h, W)[:, ::-1]                      # c -> k = 48 - c
    return np.ascontiguousarray(g)


def kernel(x, y):
    global _last_results
    from concourse.bass_utils import run_bass_kernel_spmd

    x = np.ascontiguousarray(np.asarray(x), dtype=np.float32)
    y = np.ascontiguousarray(np.asarray(y), dtype=np.float32)

    nc = build_nc()
    in_maps = _shard_inputs(x, y)
    trace = bool(int(os.environ.get("COSTVOL_TRACE", "0")))
    results = run_bass_kernel_spmd(
        nc,
        in_maps,
        core_ids=list(range(N_CORES)),
        trace=trace,
    )
    _last_results = results

    rects = np.concatenate([r["out"] for r in results.results], axis=0)
    full = _extract_band(rects)  # (16, D, H, W)
    return full.reshape(B, GROUP, D, H, W)



# revision 2
# speedup vs baseline: 1.8790x; 1.8790x over previous
"""Grouped-correlation cost volume (CostVolume) Bass kernel for Trainium2.

Problem: x, y: (4, 512, 128, 256) f32; GROUP=4, MAXDISP=48, D=49.
out[b, g, k, h, w] = sum_cg x[b, 128g+cg, h, w] * y[b, 128g+cg, h, w-k]
(zero where w < k), out shape (4, 4, 49, 128, 256).

Strategy: shard the 16 (b, g) units over 8 cores (2 each; the channel sum is
within-group, so no cross-core reduce). Per (unit, h) row the correlation is
a banded Gram matrix between x columns and y columns with contraction over
cg = 128 = the TensorE partition dim. Each 128-wide w-block is split into
four M=32 column groups (tile_position col-tiling) whose y-windows are
shifted by the group base:

  P[32m+i', 80t+j'] = sum_cg x[cg, 128t+32m+i'] * y[cg, 128t+32m-48+j']

so the useful entries are j' = i' + 48 - k with i' in [0,32), j' in [0,80) —
a 32x80 parallelogram per group (1.63x amplification instead of 3.6x for
M=128). The per-(unit,h) (128, 160) PSUM tile is cast to fp16 in SBUF and
stored to DRAM as-is; the band extraction (a pure gather) happens on the
host during the unshard step.

Precision: the whole pipeline runs in fp16 (inputs are cast on the host,
matmul accumulates in fp32 PSUM, the rect is stored as fp16). This halves
every DMA stream and quadruples TensorE throughput vs fp32; the resulting
relative error is ~1e-3, far inside the 2e-2 gate.

DMA layout: y is loaded contiguously WITHOUT the 48-column zero pad (16KB
descriptors instead of 512B rows). Windows that would read y[h, w<0] read
the previous row's tail (or SBUF slack) instead of zeros — but those
products only land in band entries with w < k, whose reference value is
exactly 0, so the host unshard step zeroes them unconditionally.

The module is built through bacc (not raw bass) so excess semaphore waits
get split onto EventSemaphore instructions — TRN2 allows at most one sync
wait per regular instruction.
"""

import os

import numpy as np

import concourse.bass as bass
import concourse.mybir as mybir
import concourse.tile as tile
from concourse import bacc

MAXDISP = 48
D = MAXDISP + 1          # 49 disparities
CG = 128                 # channels per group = contraction dim
GROUP = 4
B = 4
H = 128
W = 256
NB = W // 128            # 2 w-blocks of 128
NM = 4                   # M=32 col groups per w-block
MW = 32                  # group width
NWIN = MAXDISP + MW      # 80: y window per group
RECT_W = NB * NWIN       # 160 stored columns per (unit, h)
N_CORES = 8
N_UNITS = 2              # (b,g) units per core
H_CHUNK = 32
N_CHUNKS = H // H_CHUNK
H_PAIR = 2               # h rows per PSUM tile / copy

_last_results = None     # BassKernelResults of the most recent run (for test.py)


def build_nc(n_units=N_UNITS, n_h=H, h_chunk=H_CHUNK):
    """Build the per-core Bass module (fp16 IO, fp32 PSUM accumulate)."""
    assert n_h % h_chunk == 0
    n_chunks = n_h // h_chunk
    f16 = mybir.dt.float16
    f32 = mybir.dt.float32
    hcw = h_chunk * W

    nc = bacc.Bacc()
    x = nc.dram_tensor("x", [n_units, CG, n_h * W], f16, kind="ExternalInput")
    y = nc.dram_tensor("y", [n_units, CG, n_h * W], f16, kind="ExternalInput")
    out = nc.dram_tensor(
        "out", [n_units, n_chunks, 128, h_chunk * RECT_W], f16,
        kind="ExternalOutput",
    )

    with tile.TileContext(nc) as tc:
        with (
            tc.tile_pool(name="io", bufs=2) as io_pool,
            tc.tile_pool(name="work", bufs=2) as work_pool,
            tc.tile_pool(name="psum_mm", bufs=4, space="PSUM") as psum_mm,
        ):
            for u in range(n_units):
                for hc in range(n_chunks):
                    h0 = hc * h_chunk
                    x_tile = io_pool.tile([128, hcw], f16, name="x_tile", tag="x")
                    nc.sync.dma_start(out=x_tile, in_=x[u, :, h0 * W : h0 * W + hcw])

                    # y rows land contiguously at col 48 + h*W (48-col front
                    # slack keeps window APs non-negative; its stale contents
                    # only reach host-zeroed w<k outputs)
                    y_tile = io_pool.tile([128, 48 + hcw], f16, name="y_tile", tag="y")
                    nc.sync.dma_start(
                        out=y_tile[:, 48 : 48 + hcw],
                        in_=y[u, :, h0 * W : h0 * W + hcw],
                    )

                    # per-chunk staging tile so the store is one big DMA
                    s_big = work_pool.tile(
                        [128, h_chunk * RECT_W], f16, name="s_big", tag="S"
                    )
                    for hp in range(h_chunk // H_PAIR):
                        p_mm = psum_mm.tile(
                            [128, H_PAIR * RECT_W], f32, name="p_mm", tag="P"
                        )
                        for hh in range(H_PAIR):
                            h = hp * H_PAIR + hh
                            for t in range(NB):
                                for m in range(NM):
                                    base = 128 * t + MW * m
                                    lhsT = x_tile[:, h * W + base : h * W + base + MW]
                                    # rhs covers y cols [base-48, base+32) of
                                    # row h: tile col 48 + h*W + base - 48
                                    rhs = y_tile[
                                        :, h * W + base : h * W + base + NWIN
                                    ]
                                    nc.tensor.matmul(
                                        p_mm[
                                            MW * m : MW * (m + 1),
                                            hh * RECT_W + NWIN * t :
                                            hh * RECT_W + NWIN * (t + 1),
                                        ],
                                        lhsT,
                                        rhs,
                                        start=True,
                                        stop=True,
                                        tile_position=(0, MW * m),
                                    )
                        dst = s_big[
                            :, hp * H_PAIR * RECT_W : (hp + 1) * H_PAIR * RECT_W
                        ]
                        # alternate cast-copy between DVE and Activation
                        if hp % 2 == 0:
                            nc.vector.tensor_copy(dst, p_mm)
                        else:
                            nc.scalar.copy(dst, p_mm)
                    nc.sync.dma_start(out=out[u, hc], in_=s_big)

    nc.finalize()
    return nc


def _shard_inputs(x, y):
    """x, y: (4, 512, 128, 256) f16 -> per-core dicts of (2, 128, H*W)."""
    xu = x.reshape(B * GROUP, CG, H * W)
    yu = y.reshape(B * GROUP, CG, H * W)
    in_maps = []
    for c in range(N_CORES):
        in_maps.append(
            {
                "x": np.ascontiguousarray(xu[2 * c : 2 * c + 2]),
                "y": np.ascontiguousarray(yu[2 * c : 2 * c + 2]),
            }
        )
    return in_maps


def _extract_band(rect, n_h=H):
    """rect: (n, n_chunks, 128, h_chunk*160) rects -> (n, D, n_h, W) f32.

    rect[n, hc, 32m+i, (h'*160)+80t+j] = out[n, 48-(j-i), hc*h_chunk+h',
    128t+32m+i] for j-i in [0, 48]; entries with w < k are garbage (they
    read across y row boundaries) and are overwritten with the reference's
    exact zeros.
    """
    n, n_chunks, _, _ = rect.shape
    h_chunk = n_h // n_chunks
    r = rect.reshape(n, n_chunks, NM, MW, h_chunk, NB, NWIN)  # [n,hc,m,i,h',t,j]
    idx = np.arange(MW)[:, None] + np.arange(D)[None, :]      # j = i + c
    g = np.take_along_axis(
        r, idx[None, None, None, :, None, None, :], axis=-1
    )  # [n, hc, m, i, h', t, c]
    g = g.transpose(0, 6, 1, 4, 5, 2, 3)                      # [n,c,hc,h',t,m,i]
    g = g.reshape(n, D, n_h, W)[:, ::-1]                      # c -> k = 48 - c
    g = np.ascontiguousarray(g, dtype=np.float32)
    for k in range(1, D):                                     # out[..,k,:,w<k] = 0
        g[:, k, :, :k] = 0.0
    return g


def kernel(x, y):
    global _last_results
    from concourse.bass_utils import run_bass_kernel_spmd

    x = np.asarray(x, dtype=np.float32).astype(np.float16)
    y = np.asarray(y, dtype=np.float32).astype(np.float16)

    nc = build_nc()
    in_maps = _shard_inputs(x, y)
    trace = bool(int(os.environ.get("COSTVOL_TRACE", "0")))
    results = run_bass_kernel_spmd(
        nc,
        in_maps,
        core_ids=list(range(N_CORES)),
        trace=trace,
    )
    _last_results = results

    rects = np.concatenate([r["out"] for r in results.results], axis=0)
    full = _extract_band(rects)  # (16, D, H, W) f32
    return full.reshape(B, GROUP, D, H, W)


# revision 4
# speedup vs baseline: 1.9910x; 1.0596x over previous
"""Grouped-correlation cost volume (CostVolume) Bass kernel for Trainium2.

Problem: x, y: (4, 512, 128, 256) f32; GROUP=4, MAXDISP=48, D=49.
out[b, g, k, h, w] = sum_cg x[b, 128g+cg, h, w] * y[b, 128g+cg, h, w-k]
(zero where w < k), out shape (4, 4, 49, 128, 256).

Strategy: shard the 16 (b, g) units over 8 cores (2 each; the channel sum is
within-group, so no cross-core reduce). Per (unit, h) row the correlation is
a banded Gram matrix between x columns and y columns with contraction over
cg = 128 = the TensorE partition dim. Each 128-wide w-block is split into
four M=32 column groups (tile_position col-tiling) whose y-windows are
shifted by the group base:

  P[32m+i', 80t+j'] = sum_cg x[cg, 128t+32m+i'] * y[cg, 128t+32m-48+j']

so the useful entries are j' = i' + 48 - k with i' in [0,32), j' in [0,80) —
a 32x80 parallelogram per group (1.63x amplification instead of 3.6x for
M=128). The per-(unit,h) (128, 160) PSUM tile is cast to fp16 in SBUF and
stored to DRAM as-is; the band extraction (a pure gather) happens on the
host during the unshard step.

Precision: the whole pipeline runs in fp16 (inputs are cast on the host,
matmul accumulates in fp32 PSUM, the rect is stored as fp16). This halves
every DMA stream and quadruples TensorE throughput vs fp32; the resulting
relative error is ~1e-3, far inside the 2e-2 gate.

DMA layout: y is loaded contiguously WITHOUT the 48-column zero pad (16KB
descriptors instead of 512B rows). Windows that would read y[h, w<0] read
the previous row's tail (or SBUF slack) instead of zeros — but those
products only land in band entries with w < k, whose reference value is
exactly 0, so the host unshard step zeroes them unconditionally.

The module is built through bacc (not raw bass) so excess semaphore waits
get split onto EventSemaphore instructions — TRN2 allows at most one sync
wait per regular instruction.
"""

import os

import numpy as np

import concourse.bass as bass
import concourse.mybir as mybir
import concourse.tile as tile
from concourse import bacc

MAXDISP = 48
D = MAXDISP + 1          # 49 disparities
CG = 128                 # channels per group = contraction dim
GROUP = 4
B = 4
H = 128
W = 256
NB = W // 128            # 2 w-blocks of 128
NM = 4                   # M=32 col groups per w-block
MW = 32                  # group width
NWIN = MAXDISP + MW      # 80: y window per group
RECT_W = NB * NWIN       # 160 stored columns per (unit, h)
N_CORES = 8
N_UNITS = 2              # (b,g) units per core
H_CHUNK = 32
N_CHUNKS = H // H_CHUNK
H_PAIR = 2               # h rows per PSUM tile / copy

_last_results = None     # BassKernelResults of the most recent run (for test.py)


def build_nc(n_units=N_UNITS, n_h=H, h_chunk=H_CHUNK):
    """Build the per-core Bass module (fp16 IO, fp32 PSUM accumulate)."""
    assert n_h % h_chunk == 0
    n_chunks = n_h // h_chunk
    f16 = mybir.dt.float16
    f32 = mybir.dt.float32
    hcw = h_chunk * W

    nc = bacc.Bacc()
    x = nc.dram_tensor("x", [n_units, CG, n_h * W], f16, kind="ExternalInput")
    y = nc.dram_tensor("y", [n_units, CG, n_h * W], f16, kind="ExternalInput")
    out = nc.dram_tensor(
        "out", [n_units, n_chunks, 128, h_chunk * RECT_W], f16,
        kind="ExternalOutput",
    )

    with tile.TileContext(nc) as tc:
        with (
            tc.tile_pool(name="io", bufs=3) as io_pool,
            tc.tile_pool(name="work", bufs=3) as work_pool,
            tc.tile_pool(name="psum_mm", bufs=4, space="PSUM") as psum_mm,
        ):
            for u in range(n_units):
                for hc in range(n_chunks):
                    h0 = hc * h_chunk
                    x_tile = io_pool.tile([128, hcw], f16, name="x_tile", tag="x")
                    nc.sync.dma_start(out=x_tile, in_=x[u, :, h0 * W : h0 * W + hcw])

                    # y rows land contiguously at col 48 + h*W (48-col front
                    # slack keeps window APs non-negative; its stale contents
                    # only reach host-zeroed w<k outputs)
                    y_tile = io_pool.tile([128, 48 + hcw], f16, name="y_tile", tag="y")
                    nc.scalar.dma_start(
                        out=y_tile[:, 48 : 48 + hcw],
                        in_=y[u, :, h0 * W : h0 * W + hcw],
                    )

                    # per-chunk staging tile so the store is one big DMA
                    s_big = work_pool.tile(
                        [128, h_chunk * RECT_W], f16, name="s_big", tag="S"
                    )
                    for hp in range(h_chunk // H_PAIR):
                        p_mm = psum_mm.tile(
                            [128, H_PAIR * RECT_W], f32, name="p_mm", tag="P"
                        )
                        for hh in range(H_PAIR):
                            h = hp * H_PAIR + hh
                            for t in range(NB):
                                for m in range(NM):
                                    base = 128 * t + MW * m
                                    lhsT = x_tile[:, h * W + base : h * W + base + MW]
                                    # rhs covers y cols [base-48, base+32) of
                                    # row h: tile col 48 + h*W + base - 48
                                    rhs = y_tile[
                                        :, h * W + base : h * W + base + NWIN
                                    ]
                                    nc.tensor.matmul(
                                        p_mm[
                                            MW * m : MW * (m + 1),
                                            hh * RECT_W + NWIN * t :
                                            hh * RECT_W + NWIN * (t + 1),
                                        ],
                                        lhsT,
                                        rhs,
                                        start=True,
                                        stop=True,
                                        tile_position=(0, MW * m),
                                    )
                        dst = s_big[
                            :, hp * H_PAIR * RECT_W : (hp + 1) * H_PAIR * RECT_W
                        ]
                        # alternate cast-copy between DVE and Activation
                        if hp % 2 == 0:
                            nc.vector.tensor_copy(dst, p_mm)
                        else:
                            nc.scalar.copy(dst, p_mm)
                    nc.sync.dma_start(out=out[u, hc], in_=s_big)

    nc.finalize()
    return nc


def _shard_inputs(x, y):
    """x, y: (4, 512, 128, 256) f16 -> per-core dicts of (2, 128, H*W)."""
    xu = x.reshape(B * GROUP, CG, H * W)
    yu = y.reshape(B * GROUP, CG, H * W)
    in_maps = []
    for c in range(N_CORES):
        in_maps.append(
            {
                "x": np.ascontiguousarray(xu[2 * c : 2 * c + 2]),
                "y": np.ascontiguousarray(yu[2 * c : 2 * c + 2]),
            }
        )
    return in_maps


def _extract_band(rect, n_h=H):
    """rect: (n, n_chunks, 128, h_chunk*160) rects -> (n, D, n_h, W) f32.

    rect[n, hc, 32m+i, (h'*160)+80t+j] = out[n, 48-(j-i), hc*h_chunk+h',
    128t+32m+i] for j-i in [0, 48]; entries with w < k are garbage (they
    read across y row boundaries) and are overwritten with the reference's
    exact zeros.
    """
    n, n_chunks, _, _ = rect.shape
    h_chunk = n_h // n_chunks
    r = rect.reshape(n, n_chunks, NM, MW, h_chunk, NB, NWIN)  # [n,hc,m,i,h',t,j]
    idx = np.arange(MW)[:, None] + np.arange(D)[None, :]      # j = i + c
    g = np.take_along_axis(
        r, idx[None, None, None, :, None, None, :], axis=-1
    )  # [n, hc, m, i, h', t, c]
    g = g.transpose(0, 6, 1, 4, 5, 2, 3)                      # [n,c,hc,h',t,m,i]
    g = g.reshape(n, D, n_h, W)[:, ::-1]                      # c -> k = 48 - c
    g = np.ascontiguousarray(g, dtype=np.float32)
    for k in range(1, D):                                     # out[..,k,:,w<k] = 0
        g[:, k, :, :k] = 0.0
    return g


def kernel(x, y):
    global _last_results
    from concourse.bass_utils import run_bass_kernel_spmd

    x = np.asarray(x, dtype=np.float32).astype(np.float16)
    y = np.asarray(y, dtype=np.float32).astype(np.float16)

    nc = build_nc()
    in_maps = _shard_inputs(x, y)
    trace = bool(int(os.environ.get("COSTVOL_TRACE", "0")))
    results = run_bass_kernel_spmd(
        nc,
        in_maps,
        core_ids=list(range(N_CORES)),
        trace=trace,
    )
    _last_results = results

    rects = np.concatenate([r["out"] for r in results.results], axis=0)
    full = _extract_band(rects)  # (16, D, H, W) f32
    return full.reshape(B, GROUP, D, H, W)


# revision 6
# speedup vs baseline: 2.0212x; 1.0152x over previous
"""Grouped-correlation cost volume (CostVolume) Bass kernel for Trainium2.

Problem: x, y: (4, 512, 128, 256) f32; GROUP=4, MAXDISP=48, D=49.
out[b, g, k, h, w] = sum_cg x[b, 128g+cg, h, w] * y[b, 128g+cg, h, w-k]
(zero where w < k), out shape (4, 4, 49, 128, 256).

Strategy: shard the 16 (b, g) units over 8 cores (2 each; the channel sum is
within-group, so no cross-core reduce). Per (unit, h) row the correlation is
a banded Gram matrix between x columns and y columns with contraction over
cg = 128 = the TensorE partition dim. Each 128-wide w-block is split into
four M=32 column groups (tile_position col-tiling) whose y-windows are
shifted by the group base:

  P[32m+i', 80t+j'] = sum_cg x[cg, 128t+32m+i'] * y[cg, 128t+32m-48+j']

so the useful entries are j' = i' + 48 - k with i' in [0,32), j' in [0,80) —
a 32x80 parallelogram per group (1.63x amplification instead of 3.6x for
M=128). The per-(unit,h) (128, 160) PSUM tile is cast to fp16 in SBUF and
stored to DRAM as-is; the band extraction (a pure gather) happens on the
host during the unshard step.

Precision: the whole pipeline runs in fp16 (inputs are cast on the host,
matmul accumulates in fp32 PSUM, the rect is stored as fp16). This halves
every DMA stream and quadruples TensorE throughput vs fp32; the resulting
relative error is ~1e-3, far inside the 2e-2 gate.

DMA layout: y is loaded contiguously WITHOUT the 48-column zero pad (16KB
descriptors instead of 512B rows). Windows that would read y[h, w<0] read
the previous row's tail (or SBUF slack) instead of zeros — but those
products only land in band entries with w < k, whose reference value is
exactly 0, so the host unshard step zeroes them unconditionally.

The module is built through bacc (not raw bass) so excess semaphore waits
get split onto EventSemaphore instructions — TRN2 allows at most one sync
wait per regular instruction.
"""

import os

import numpy as np

import concourse.bass as bass
import concourse.mybir as mybir
import concourse.tile as tile
from concourse import bacc

MAXDISP = 48
D = MAXDISP + 1          # 49 disparities
CG = 128                 # channels per group = contraction dim
GROUP = 4
B = 4
H = 128
W = 256
NB = W // 128            # 2 w-blocks of 128
NM = 4                   # M=32 col groups per w-block
MW = 32                  # group width
NWIN = MAXDISP + MW      # 80: y window per group
RECT_W = NB * NWIN       # 160 stored columns per (unit, h)
N_CORES = 8
N_UNITS = 2              # (b,g) units per core
H_CHUNK = 32
N_CHUNKS = H // H_CHUNK
H_PAIR = 2               # h rows per PSUM tile / copy

_last_results = None     # BassKernelResults of the most recent run (for test.py)


def build_nc(n_units=N_UNITS, n_h=H, h_chunk=H_CHUNK):
    """Build the per-core Bass module (fp16 IO, fp32 PSUM accumulate)."""
    assert n_h % h_chunk == 0
    n_chunks = n_h // h_chunk
    f16 = mybir.dt.float16
    f32 = mybir.dt.float32
    hcw = h_chunk * W

    nc = bacc.Bacc()
    x = nc.dram_tensor("x", [n_units, CG, n_h * W], f16, kind="ExternalInput")
    y = nc.dram_tensor("y", [n_units, CG, n_h * W], f16, kind="ExternalInput")
    out = nc.dram_tensor(
        "out", [n_units, n_chunks, 128, h_chunk * RECT_W], f16,
        kind="ExternalOutput",
    )

    with tile.TileContext(nc) as tc:
        with (
            tc.tile_pool(name="io", bufs=3) as io_pool,
            tc.tile_pool(name="work", bufs=3) as work_pool,
            tc.tile_pool(name="psum_mm", bufs=4, space="PSUM") as psum_mm,
        ):
            for u in range(n_units):
                for hc in range(n_chunks):
                    h0 = hc * h_chunk
                    x_tile = io_pool.tile([128, hcw], f16, name="x_tile", tag="x")
                    nc.sync.dma_start(out=x_tile, in_=x[u, :, h0 * W : h0 * W + hcw])

                    # y rows land contiguously at col 48 + h*W (48-col front
                    # slack keeps window APs non-negative; its stale contents
                    # only reach host-zeroed w<k outputs)
                    y_tile = io_pool.tile([128, 48 + hcw], f16, name="y_tile", tag="y")
                    nc.sync.dma_start(
                        out=y_tile[:, 48 : 48 + hcw],
                        in_=y[u, :, h0 * W : h0 * W + hcw],
                    )

                    # per-chunk staging tile so the store is one big DMA
                    s_big = work_pool.tile(
                        [128, h_chunk * RECT_W], f16, name="s_big", tag="S"
                    )
                    for hp in range(h_chunk // H_PAIR):
                        p_mm = psum_mm.tile(
                            [128, H_PAIR * RECT_W], f32, name="p_mm", tag="P"
                        )
                        for hh in range(H_PAIR):
                            h = hp * H_PAIR + hh
                            for t in range(NB):
                                for m in range(NM):
                                    base = 128 * t + MW * m
                                    lhsT = x_tile[:, h * W + base : h * W + base + MW]
                                    # rhs covers y cols [base-48, base+32) of
                                    # row h: tile col 48 + h*W + base - 48
                                    rhs = y_tile[
                                        :, h * W + base : h * W + base + NWIN
                                    ]
                                    nc.tensor.matmul(
                                        p_mm[
                                            MW * m : MW * (m + 1),
                                            hh * RECT_W + NWIN * t :
                                            hh * RECT_W + NWIN * (t + 1),
                                        ],
                                        lhsT,
                                        rhs,
                                        start=True,
                                        stop=True,
                                        tile_position=(0, MW * m),
                                    )
                        dst = s_big[
                            :, hp * H_PAIR * RECT_W : (hp + 1) * H_PAIR * RECT_W
                        ]
                        # alternate cast-copy between DVE and Activation
                        if hp % 2 == 0:
                            nc.vector.tensor_copy(dst, p_mm)
                        else:
                            nc.scalar.copy(dst, p_mm)
                    # stores on their own engine: an in-order sequencer that
                    # also issued loads would stall them behind store waits
                    nc.gpsimd.dma_start(out=out[u, hc], in_=s_big)

    nc.finalize()
    return nc


def _shard_inputs(x, y):
    """x, y: (4, 512, 128, 256) f16 -> per-core dicts of (2, 128, H*W)."""
    xu = x.reshape(B * GROUP, CG, H * W)
    yu = y.reshape(B * GROUP, CG, H * W)
    in_maps = []
    for c in range(N_CORES):
        in_maps.append(
            {
                "x": np.ascontiguousarray(xu[2 * c : 2 * c + 2]),
                "y": np.ascontiguousarray(yu[2 * c : 2 * c + 2]),
            }
        )
    return in_maps


def _extract_band(rect, n_h=H):
    """rect: (n, n_chunks, 128, h_chunk*160) rects -> (n, D, n_h, W) f32.

    rect[n, hc, 32m+i, (h'*160)+80t+j] = out[n, 48-(j-i), hc*h_chunk+h',
    128t+32m+i] for j-i in [0, 48]; entries with w < k are garbage (they
    read across y row boundaries) and are overwritten with the reference's
    exact zeros.
    """
    n, n_chunks, _, _ = rect.shape
    h_chunk = n_h // n_chunks
    r = rect.reshape(n, n_chunks, NM, MW, h_chunk, NB, NWIN)  # [n,hc,m,i,h',t,j]
    idx = np.arange(MW)[:, None] + np.arange(D)[None, :]      # j = i + c
    g = np.take_along_axis(
        r, idx[None, None, None, :, None, None, :], axis=-1
    )  # [n, hc, m, i, h', t, c]
    g = g.transpose(0, 6, 1, 4, 5, 2, 3)                      # [n,c,hc,h',t,m,i]
    g = g.reshape(n, D, n_h, W)[:, ::-1]                      # c -> k = 48 - c
    g = np.ascontiguousarray(g, dtype=np.float32)
    for k in range(1, D):                                     # out[..,k,:,w<k] = 0
        g[:, k, :, :k] = 0.0
    return g


def kernel(x, y):
    global _last_results
    from concourse.bass_utils import run_bass_kernel_spmd

    x = np.asarray(x, dtype=np.float32).astype(np.float16)
    y = np.asarray(y, dtype=np.float32).astype(np.float16)

    nc = build_nc()
    in_maps = _shard_inputs(x, y)
    trace = bool(int(os.environ.get("COSTVOL_TRACE", "0")))
    results = run_bass_kernel_spmd(
        nc,
        in_maps,
        core_ids=list(range(N_CORES)),
        trace=trace,
    )
    _last_results = results

    rects = np.concatenate([r["out"] for r in results.results], axis=0)
    full = _extract_band(rects)  # (16, D, H, W) f32
    return full.reshape(B, GROUP, D, H, W)


# revision 8
# speedup vs baseline: 2.0733x; 1.0258x over previous
"""Grouped-correlation cost volume (CostVolume) Bass kernel for Trainium2.

Problem: x, y: (4, 512, 128, 256) f32; GROUP=4, MAXDISP=48, D=49.
out[b, g, k, h, w] = sum_cg x[b, 128g+cg, h, w] * y[b, 128g+cg, h, w-k]
(zero where w < k), out shape (4, 4, 49, 128, 256).

Strategy: shard the 16 (b, g) units over 8 cores (2 each; the channel sum is
within-group, so no cross-core reduce). Per (unit, h) row the correlation is
a banded Gram matrix between x columns and y columns with contraction over
cg = 128 = the TensorE partition dim. Each 128-wide w-block is split into
four M=32 column groups (tile_position col-tiling) whose y-windows are
shifted by the group base:

  P[32m+i', 80t+j'] = sum_cg x[cg, 128t+32m+i'] * y[cg, 128t+32m-48+j']

so the useful entries are j' = i' + 48 - k with i' in [0,32), j' in [0,80) —
a 32x80 parallelogram per group (1.63x amplification instead of 3.6x for
M=128). The per-(unit,h) (128, 160) PSUM rows are cast to fp16 in SBUF and
stored to DRAM as-is; the band extraction (a pure gather) happens on the
host during the unshard step.

Precision: the whole pipeline runs in fp16 (inputs are cast on the host,
matmul accumulates in fp32 PSUM, the rect is stored as fp16). This halves
every DMA stream and quadruples TensorE throughput vs fp32; the resulting
relative error is ~4e-4, far inside the 2e-2 gate.

DMA layout: y is loaded contiguously WITHOUT the 48-column zero pad (16KB
descriptors instead of 512B rows). Windows that would read y[h, w<0] read
the previous row's tail (or SBUF slack) instead of zeros — but those
products only land in band entries with w < k, whose reference value is
exactly 0, so the host unshard step zeroes them unconditionally.

Scheduling: x loads issue from SP, y loads from Activation, stores from
Pool — each stream on its own in-order sequencer so none stalls another
(a sequencer that issued loads and stores would hold the next load behind
the store's semaphore wait). PSUM->SBUF cast-copies alternate DVE /
Activation. The very last chunk's store is split into 8-row pieces that
fire as their copies complete, shortening the post-final-load tail.

The module is built through bacc (not raw bass) so excess semaphore waits
get split onto EventSemaphore instructions — TRN2 allows at most one sync
wait per regular instruction.
"""

import os

import numpy as np

import concourse.bass as bass
import concourse.mybir as mybir
import concourse.tile as tile
from concourse import bacc

MAXDISP = 48
D = MAXDISP + 1          # 49 disparities
CG = 128                 # channels per group = contraction dim
GROUP = 4
B = 4
H = 128
W = 256
NB = W // 128            # 2 w-blocks of 128
NM = 4                   # M=32 col groups per w-block
MW = 32                  # group width
NWIN = MAXDISP + MW      # 80: y window per group
RECT_W = NB * NWIN       # 160 stored columns per (unit, h)
N_CORES = 8
N_UNITS = 2              # (b,g) units per core
H_CHUNK = 32
N_CHUNKS = H // H_CHUNK
H_PAIR = 2               # h rows per PSUM tile / copy

_last_results = None     # BassKernelResults of the most recent run (for test.py)


def build_nc(n_units=N_UNITS, n_h=H, h_chunk=H_CHUNK):
    """Build the per-core Bass module (fp16 IO, fp32 PSUM accumulate)."""
    assert n_h % h_chunk == 0
    n_chunks = n_h // h_chunk
    f16 = mybir.dt.float16
    f32 = mybir.dt.float32
    hcw = h_chunk * W

    nc = bacc.Bacc()
    x = nc.dram_tensor("x", [n_units, CG, n_h * W], f16, kind="ExternalInput")
    y = nc.dram_tensor("y", [n_units, CG, n_h * W], f16, kind="ExternalInput")
    out = nc.dram_tensor(
        "out", [n_units, n_chunks, 128, h_chunk * RECT_W], f16,
        kind="ExternalOutput",
    )

    with tile.TileContext(nc) as tc:
        with (
            tc.tile_pool(name="io", bufs=3) as io_pool,
            tc.tile_pool(name="work", bufs=3) as work_pool,
            tc.tile_pool(name="psum_mm", bufs=4, space="PSUM") as psum_mm,
        ):
            for u in range(n_units):
                for hc in range(n_chunks):
                    h0 = hc * h_chunk
                    last_chunk = u == n_units - 1 and hc == n_chunks - 1
                    x_tile = io_pool.tile([128, hcw], f16, name="x_tile", tag="x")
                    nc.sync.dma_start(out=x_tile, in_=x[u, :, h0 * W : h0 * W + hcw])

                    # y rows land contiguously at col 48 + h*W (48-col front
                    # slack keeps window APs non-negative; its stale contents
                    # only reach host-zeroed w<k outputs)
                    y_tile = io_pool.tile([128, 48 + hcw], f16, name="y_tile", tag="y")
                    nc.scalar.dma_start(
                        out=y_tile[:, 48 : 48 + hcw],
                        in_=y[u, :, h0 * W : h0 * W + hcw],
                    )

                    # per-chunk staging tile so the store is one big DMA
                    s_big = work_pool.tile(
                        [128, h_chunk * RECT_W], f16, name="s_big", tag="S"
                    )
                    for hp in range(h_chunk // H_PAIR):
                        p_mm = psum_mm.tile(
                            [128, H_PAIR * RECT_W], f32, name="p_mm", tag="P"
                        )
                        for hh in range(H_PAIR):
                            h = hp * H_PAIR + hh
                            for t in range(NB):
                                for m in range(NM):
                                    base = 128 * t + MW * m
                                    lhsT = x_tile[:, h * W + base : h * W + base + MW]
                                    # rhs covers y cols [base-48, base+32) of
                                    # row h: tile col 48 + h*W + base - 48
                                    rhs = y_tile[
                                        :, h * W + base : h * W + base + NWIN
                                    ]
                                    nc.tensor.matmul(
                                        p_mm[
                                            MW * m : MW * (m + 1),
                                            hh * RECT_W + NWIN * t :
                                            hh * RECT_W + NWIN * (t + 1),
                                        ],
                                        lhsT,
                                        rhs,
                                        start=True,
                                        stop=True,
                                        tile_position=(0, MW * m),
                                    )
                        dst = s_big[
                            :, hp * H_PAIR * RECT_W : (hp + 1) * H_PAIR * RECT_W
                        ]
                        # alternate cast-copy between DVE and Activation
                        if hp % 2 == 0:
                            nc.vector.tensor_copy(dst, p_mm)
                        else:
                            nc.scalar.copy(dst, p_mm)
                        # tail taper: the final chunk streams out in 8-row
                        # pieces as copies land instead of one store at the
                        # end, so little work remains after the last load
                        if last_chunk and hp % 4 == 3:
                            q0 = (hp - 3) * H_PAIR * RECT_W
                            q1 = (hp + 1) * H_PAIR * RECT_W
                            nc.gpsimd.dma_start(
                                out=out[u, hc][:, q0:q1], in_=s_big[:, q0:q1]
                            )
                    # stores on their own engine: an in-order sequencer that
                    # also issued loads would stall them behind store waits
                    if not last_chunk:
                        nc.gpsimd.dma_start(out=out[u, hc], in_=s_big)

    nc.finalize()
    return nc


def _shard_inputs(x, y):
    """x, y: (4, 512, 128, 256) f16 -> per-core dicts of (2, 128, H*W)."""
    xu = x.reshape(B * GROUP, CG, H * W)
    yu = y.reshape(B * GROUP, CG, H * W)
    in_maps = []
    for c in range(N_CORES):
        in_maps.append(
            {
                "x": np.ascontiguousarray(xu[2 * c : 2 * c + 2]),
                "y": np.ascontiguousarray(yu[2 * c : 2 * c + 2]),
            }
        )
    return in_maps


def _extract_band(rect, n_h=H):
    """rect: (n, n_chunks, 128, h_chunk*160) rects -> (n, D, n_h, W) f32.

    rect[n, hc, 32m+i, (h'*160)+80t+j] = out[n, 48-(j-i), hc*h_chunk+h',
    128t+32m+i] for j-i in [0, 48]; entries with w < k are garbage (they
    read across y row boundaries) and are overwritten with the reference's
    exact zeros.
    """
    n, n_chunks, _, _ = rect.shape
    h_chunk = n_h // n_chunks
    r = rect.reshape(n, n_chunks, NM, MW, h_chunk, NB, NWIN)  # [n,hc,m,i,h',t,j]
    idx = np.arange(MW)[:, None] + np.arange(D)[None, :]      # j = i + c
    g = np.take_along_axis(
        r, idx[None, None, None, :, None, None, :], axis=-1
    )  # [n, hc, m, i, h', t, c]
    g = g.transpose(0, 6, 1, 4, 5, 2, 3)                      # [n,c,hc,h',t,m,i]
    g = g.reshape(n, D, n_h, W)[:, ::-1]                      # c -> k = 48 - c
    g = np.ascontiguousarray(g, dtype=np.float32)
    for k in range(1, D):                                     # out[..,k,:,w<k] = 0
        g[:, k, :, :k] = 0.0
    return g


def kernel(x, y):
    global _last_results
    from concourse.bass_utils import run_bass_kernel_spmd

    x = np.asarray(x, dtype=np.float32).astype(np.float16)
    y = np.asarray(y, dtype=np.float32).astype(np.float16)

    nc = build_nc()
    in_maps = _shard_inputs(x, y)
    trace = bool(int(os.environ.get("COSTVOL_TRACE", "0")))
    results = run_bass_kernel_spmd(
        nc,
        in_maps,
        core_ids=list(range(N_CORES)),
        trace=trace,
    )
    _last_results = results

    rects = np.concatenate([r["out"] for r in results.results], axis=0)
    full = _extract_band(rects)  # (16, D, H, W) f32
    return full.reshape(B, GROUP, D, H, W)
